# revision 16
# baseline (speedup 1.0000x reference)
"""CG-SENSE MRI reconstruction (nn_CGClass) on 8 Trainium2 NeuronCores.

Strategy: data-parallel over batch (B=8 -> 1 sample per core). Per core the
whole CG (10 iterations) runs on-chip. fft2/ifft2 are dense 320-point DFTs
done on the tensor engine as fp16 matmuls via the transpose-free primitive
OUT = Z^T @ A (data stationary, DFT matrix moving); applying it twice gives
F @ Z @ F with no transposes. CG state stays fp32; matmul operands and the
elementwise complex products are fp16 (DVE 2x mode). The CG scalar phase
uses the Gauss recurrence rTrNew = rTr - alpha*(2*pAp - alpha*ApAp) so both
dots are available as soon as Ap is, and r/p/x updates are single fused
scalar_tensor_tensor ops. Small matmuls chained on scalar-phase
intermediates keep the PE HAM clock warm across the serial section.

Layout: each 320x320 real array lives in SBUF as [128, 960]: free-dim block
t in {0,1,2} holds image rows [128t : 128t+{128,128,64}]. Block 2 uses
partitions 0..63; pad regions are kept zero (NaN hygiene for reductions).
NOTE: matmuls with lhsT/rhs at base_partition 64 hang this hardware path
(verified 3x) — all stationary/moving operands must stay at base 0.
"""
import os
from contextlib import ExitStack

import numpy as np

import concourse.bass as bass  # noqa: F401
import concourse.tile as tile
from concourse import mybir, bass_utils, bacc

F32 = mybir.dt.float32
F16 = mybir.dt.float16
MULT = mybir.AluOpType.mult
ADD = mybir.AluOpType.add
SUB = mybir.AluOpType.subtract

H = 320
B, C = 8, 12
N_ITER = int(os.environ.get("CG_ITERS", "10"))
KT = [(0, 128), (128, 128), (256, 64)]  # (row_start, rows) per block

_PROGRAM = None
TRACE = bool(os.environ.get("CG_TRACE"))
UNROLL = bool(os.environ.get("CG_UNROLL"))


def _mblk(t):
    return slice(320 * t, 320 * t + 320)


def _build_consts():
    j = np.arange(H)
    ang = -2.0 * np.pi * np.outer(j, j) / H
    scale = 1.0 / np.sqrt(H)
    Fr = (np.cos(ang) * scale).astype(np.float32)
    Fi = (np.sin(ang) * scale).astype(np.float32)

    def blocks(a):
        out = np.zeros((128, 960), np.float32)
        for t, (s, sz) in enumerate(KT):
            out[:sz, 320 * t:320 * t + 320] = a[s:s + sz]
        return out

    return {
        "c_fr": blocks(Fr).astype(np.float16),
        "c_fi": blocks(Fi).astype(np.float16),
        "c_nfi": blocks(-Fi).astype(np.float16),
        "ones_col": np.ones((128, 1), np.float32),
        "ones_row": np.ones((1, 128), np.float32),
    }


def _build_program():
    nc = bacc.Bacc("TRN2", target_bir_lowering=False, debug=False)

    d = {}
    d["x_re"] = nc.dram_tensor("x_re", [H, H], F32, kind="ExternalInput")
    d["x_im"] = nc.dram_tensor("x_im", [H, H], F32, kind="ExternalInput")
    d["y_re"] = nc.dram_tensor("y_re", [C, H, H], F32, kind="ExternalInput")
    d["y_im"] = nc.dram_tensor("y_im", [C, H, H], F32, kind="ExternalInput")
    d["s_re"] = nc.dram_tensor("s_re", [C, H, H], F16, kind="ExternalInput")
    d["s_im"] = nc.dram_tensor("s_im", [C, H, H], F16, kind="ExternalInput")
    d["mask"] = nc.dram_tensor("mask", [H, H], F32, kind="ExternalInput")
    d["lam_b"] = nc.dram_tensor("lam_b", [128, 1], F32, kind="ExternalInput")
    d["c_fr"] = nc.dram_tensor("c_fr", [128, 960], F16, kind="ExternalInput")
    d["c_fi"] = nc.dram_tensor("c_fi", [128, 960], F16, kind="ExternalInput")
    d["c_nfi"] = nc.dram_tensor("c_nfi", [128, 960], F16, kind="ExternalInput")
    d["ones_col"] = nc.dram_tensor("ones_col", [128, 1], F32, kind="ExternalInput")
    d["ones_row"] = nc.dram_tensor("ones_row", [1, 128], F32, kind="ExternalInput")
    d["out"] = nc.dram_tensor("out", [2, H, H], F32, kind="ExternalOutput")

    with tile.TileContext(nc) as tc, ExitStack() as ctx:
        persist = ctx.enter_context(tc.tile_pool(name="persist", bufs=1))
        stg16 = ctx.enter_context(tc.tile_pool(name="stg16", bufs=2))
        tmp32 = ctx.enter_context(tc.tile_pool(name="tmp32", bufs=2))
        tmp16 = ctx.enter_context(tc.tile_pool(name="tmp16", bufs=2))
        ps = ctx.enter_context(tc.tile_pool(name="ps", bufs=3, space="PSUM"))
        pss = ctx.enter_context(tc.tile_pool(name="pss", bufs=1, space="PSUM"))

        def load_blocks(dst, src_ap):
            for t, (s, sz) in enumerate(KT):
                nc.sync.dma_start(dst[0:sz, _mblk(t)], src_ap[s:s + sz, :])

        def zero_pad(t32, eng=None):
            (eng or nc.vector).memset(t32[64:128, 640:960], 0.0)

        # ---- persistent tiles ----
        sr = [persist.tile([128, 960], F16, tag=f"sr{c}", name=f"sr{c}") for c in range(C)]
        si = [persist.tile([128, 960], F16, tag=f"si{c}", name=f"si{c}") for c in range(C)]
        mask2 = persist.tile([128, 960], F32, tag="mask2", name="mask2")
        mask1 = persist.tile([128, 960], F32, tag="mask1", name="mask1")
        p_re = persist.tile([128, 960], F32, tag="p_re", name="p_re")
        p_im = persist.tile([128, 960], F32, tag="p_im", name="p_im")
        p16r = persist.tile([128, 960], F16, tag="p16r", name="p16r")
        p16i = persist.tile([128, 960], F16, tag="p16i", name="p16i")
        r_re = persist.tile([128, 960], F32, tag="r_re", name="r_re")
        r_im = persist.tile([128, 960], F32, tag="r_im", name="r_im")
        x_re = persist.tile([128, 960], F32, tag="x_re", name="x_re")
        x_im = persist.tile([128, 960], F32, tag="x_im", name="x_im")
        ap_re = persist.tile([128, 960], F32, tag="ap_re", name="ap_re")
        ap_im = persist.tile([128, 960], F32, tag="ap_im", name="ap_im")
        fr = persist.tile([128, 960], F16, tag="fr", name="fr")
        fi = persist.tile([128, 960], F16, tag="fi", name="fi")
        nfi = persist.tile([128, 960], F16, tag="nfi", name="nfi")
        ones_col = persist.tile([128, 1], F32, tag="ones_col", name="ones_col")
        ones_row = persist.tile([1, 128], F32, tag="ones_row", name="ones_row")
        lam_b = persist.tile([128, 1], F32, tag="lam_b", name="lam_b")
        ab = persist.tile([128, 2], F32, tag="ab", name="ab")  # col0=alpha col1=-alpha
        beta_b = persist.tile([128, 1], F32, tag="beta_b", name="beta_b")
        dacc = persist.tile([128, 4], F32, tag="dacc", name="dacc")
        sc = persist.tile([1, 12], F32, tag="sc", name="sc")
        # sc slots: 0=rTr 1=inv_rTr 2=pAp 3=alpha 4=-alpha 5=beta 6=rTrNew
        # 7..10 reduce tmps, 10=ApAp via 8+9, 11=tmp
        sp0r = persist.tile([128, 960], F16, tag="sp0r", name="sp0r")
        sp0i = persist.tile([128, 960], F16, tag="sp0i", name="sp0i")
        scr = persist.tile([128, 960], F32, tag="scr", name="scr")
        scr2 = persist.tile([128, 960], F32, tag="scr2", name="scr2")
        jnk = persist.tile([128, 960], F32, tag="jnk", name="jnk")

        def emit_side(zr16, zi16, chain, consume, kt_major=False):
            """psum(re,im) per m-block of Z^T @ A, complex. consume(mt,msz,pre,pim).

            kt_major=True issues the K-blocks outermost so the first matmuls
            need only block 0 of the input — lets the serial-phase tail feed
            the PE block-by-block. Uses 6 live psum banks."""
            if chain == "F":
                mov = [(zr16, fr, "re"), (zr16, fi, "im"),
                       (zi16, nfi, "re"), (zi16, fr, "im")]
            else:  # G = conj(F)
                mov = [(zr16, fr, "re"), (zr16, nfi, "im"),
                       (zi16, fi, "re"), (zi16, fr, "im")]
            if kt_major:
                pres, pims, cnts = [], [], []
                for mt in range(3):
                    pres.append(ps.tile([128, 320], F32, tag="ps_re",
                                        name="ps_re", bufs=4))
                    pims.append(ps.tile([128, 320], F32, tag="ps_im",
                                        name="ps_im"))
                    cnts.append({"re": 0, "im": 0})
                for kt, (ks, ksz) in enumerate(KT):
                    for mt, (ms, msz) in enumerate(KT):
                        for z, a, dst in mov:
                            lo = 320 * kt + 128 * mt
                            pt = (pres[mt] if dst == "re" else pims[mt])[0:msz, :]
                            cnts[mt][dst] += 1
                            nc.tensor.matmul(pt, z[0:ksz, lo:lo + msz],
                                             a[0:ksz, _mblk(kt)],
                                             start=(cnts[mt][dst] == 1),
                                             stop=(cnts[mt][dst] == 6))
                for mt, (ms, msz) in enumerate(KT):
                    consume(mt, msz, pres[mt], pims[mt])
                return
            for mt, (ms, msz) in enumerate(KT):
                pre = ps.tile([128, 320], F32, tag="ps_re", name="ps_re", bufs=4)
                pim = ps.tile([128, 960], F32, tag="ps_im", name="ps_im") if False else ps.tile([128, 320], F32, tag="ps_im", name="ps_im")
                cnt = {"re": 0, "im": 0}
                for kt, (ks, ksz) in enumerate(KT):
                    for z, a, dst in mov:
                        lo = 320 * kt + 128 * mt
                        pt = (pre if dst == "re" else pim)[0:msz, :]
                        cnt[dst] += 1
                        nc.tensor.matmul(pt, z[0:ksz, lo:lo + msz],
                                         a[0:ksz, _mblk(kt)],
                                         start=(cnt[dst] == 1),
                                         stop=(cnt[dst] == 6))
                consume(mt, msz, pre, pim)

        def make_sp(c, blocked=False):
            """spr + i*spi = (s_c) * (p), fp16 elementwise on DVE.
            blocked=True emits per 320-col block so block 0 is ready early
            (feeds the kt-major stage-1 right after the p-update)."""
            spr = stg16.tile([128, 960], F16, tag="spr", name="spr")
            spi = stg16.tile([128, 960], F16, tag="spi", name="spi")
            t1 = tmp16.tile([128, 960], F16, tag="mm_t1", name="mm_t1")
            t2 = tmp16.tile([128, 960], F16, tag="mm_t2", name="mm_t2")
            t3 = tmp16.tile([128, 960], F16, tag="mm_t3", name="mm_t3")
            t4 = tmp16.tile([128, 960], F16, tag="mm_t4", name="mm_t4")
            blks = [slice(320 * t, 320 * t + 320) for t in range(3)] \
                if blocked else [slice(0, 960)]
            for b in blks:
                nc.vector.tensor_mul(t1[:, b], sr[c][:, b], p16r[:, b])
                nc.vector.tensor_mul(t2[:, b], si[c][:, b], p16i[:, b])
                nc.vector.tensor_mul(t3[:, b], sr[c][:, b], p16i[:, b])
                nc.vector.tensor_mul(t4[:, b], si[c][:, b], p16r[:, b])
                nc.vector.tensor_sub(spr[:, b], t1[:, b], t2[:, b])
                nc.vector.tensor_add(spi[:, b], t3[:, b], t4[:, b])
            return spr, spi

        def combine_coil(c, u4r, u4i, last=False):
            """ap += conj(s_c) * u4. im-chain on GpSimd except for the last
            coil, whose result gates the serial-phase dots."""
            t1 = tmp16.tile([128, 960], F16, tag="cb_t1", name="cb_t1")
            t2 = tmp16.tile([128, 960], F16, tag="cb_t2", name="cb_t2")
            t3 = tmp16.tile([128, 960], F16, tag="cb_t3", name="cb_t3")
            t4 = tmp16.tile([128, 960], F16, tag="cb_t4", name="cb_t4")
            nc.vector.tensor_mul(t1[:], sr[c][:], u4r[:])
            nc.vector.tensor_mul(t2[:], si[c][:], u4i[:])
            nc.vector.tensor_mul(t3[:], sr[c][:], u4i[:])
            nc.vector.tensor_mul(t4[:], si[c][:], u4r[:])
            nc.vector.tensor_add(ap_re[:], ap_re[:], t1[:])
            nc.vector.tensor_add(ap_re[:], ap_re[:], t2[:])
            eng = nc.vector if last else nc.gpsimd
            eng.tensor_add(ap_im[:], ap_im[:], t3[:])
            eng.tensor_sub(ap_im[:], ap_im[:], t4[:])

        def ifft_and_combine(c, inr16, ini16):
            """Emit S3/S4 G-chain; return deferred combine closure."""
            s3r = stg16.tile([128, 960], F16, tag="s3r", name="s3r")
            s3i = stg16.tile([128, 960], F16, tag="s3i", name="s3i")

            def consume3(mt, msz, pre, pim):
                nc.scalar.copy(s3r[0:msz, _mblk(mt)], pre[0:msz, :])
                nc.scalar.copy(s3i[0:msz, _mblk(mt)], pim[0:msz, :])
            emit_side(inr16, ini16, "G", consume3)

            u4r = tmp16.tile([128, 960], F16, tag="u4r", name="u4r")
            u4i = tmp16.tile([128, 960], F16, tag="u4i", name="u4i")

            def consume4(mt, msz, pre, pim):
                nc.scalar.copy(u4r[0:msz, _mblk(mt)], pre[0:msz, :])
                nc.scalar.copy(u4i[0:msz, _mblk(mt)], pim[0:msz, :])
            emit_side(s3r, s3i, "G", consume4)
            return lambda: combine_coil(c, u4r, u4i, last=(c == C - 1))

        # single shared PSUM bank for all small matmul outputs (PSUM tiles
        # are bank-granular; disjoint column slices of one tile instead)
        pmisc = pss.tile([128, 384], F32, tag="pmisc", name="pmisc")

        def reduce_dacc(col0, ncols, dst_slot):
            """sc[0, dst_slot:...] = per-col partition sums of dacc cols."""
            nc.tensor.matmul(pmisc[0:1, 0:ncols], ones_col[:, 0:1],
                             dacc[:, col0:col0 + ncols], start=True, stop=True)
            nc.vector.tensor_copy(sc[0:1, dst_slot:dst_slot + ncols],
                                  pmisc[0:1, 0:ncols])

        def warm_mm(dep32):
            """PE keep-warm matmul chained on a scalar-phase intermediate."""
            nc.tensor.matmul(pmisc[0:1, 64:384], dep32[:, 0:1],
                             mask2[:, 0:320], start=True, stop=True)

        def warm_chain(dep32, n):
            for _ in range(n):
                warm_mm(dep32)

        # ---- load constants + inputs ----
        nc.sync.dma_start(fr[:], d["c_fr"].ap())
        nc.sync.dma_start(fi[:], d["c_fi"].ap())
        nc.sync.dma_start(nfi[:], d["c_nfi"].ap())
        nc.sync.dma_start(ones_col[:], d["ones_col"].ap())
        nc.sync.dma_start(ones_row[:], d["ones_row"].ap())
        nc.sync.dma_start(lam_b[:], d["lam_b"].ap())
        load_blocks(mask1, d["mask"].ap())
        zero_pad(mask1)
        nc.vector.tensor_mul(mask2[:], mask1[:], mask1[:])
        for c in range(C):
            load_blocks(sr[c], d["s_re"].ap()[c])
            load_blocks(si[c], d["s_im"].ap()[c])
            zero_pad(sr[c])
            zero_pad(si[c])
        load_blocks(x_re, d["x_re"].ap())
        load_blocks(x_im, d["x_im"].ap())
        zero_pad(x_re)
        zero_pad(x_im)

        nc.vector.memset(ap_re[:], 0.0)
        nc.vector.memset(ap_im[:], 0.0)
        # one-time pad hygiene for pool-cycled fp16 tmp tiles (pads never
        # rewritten afterwards; products with zero-padded sr/si stay finite)
        for _ in range(2):
            for tg in ("mm_t1", "mm_t2", "mm_t3", "mm_t4",
                       "cb_t1", "cb_t2", "cb_t3", "cb_t4", "u4r", "u4i"):
                t = tmp16.tile([128, 960], F16, tag=tg, name=tg)
                nc.gpsimd.memset(t[64:128, 640:960], 0.0)
        for _ in range(2):
            for tg in ("spr", "spi", "s1r", "s1i", "wr", "wi", "s3r", "s3i"):
                t = stg16.tile([128, 960], F16, tag=tg, name=tg)
                nc.gpsimd.memset(t[64:128, 640:960], 0.0)

        # ---- phase 1: rhs ----
        def make_my(c):
            yr = tmp32.tile([128, 960], F32, tag="yr", name="yr")
            yi = tmp32.tile([128, 960], F32, tag="yi", name="yi")
            load_blocks(yr, d["y_re"].ap()[c])
            load_blocks(yi, d["y_im"].ap()[c])
            zero_pad(yr, nc.gpsimd)
            zero_pad(yi, nc.gpsimd)
            myr = stg16.tile([128, 960], F16, tag="spr", name="myr")
            myi = stg16.tile([128, 960], F16, tag="spi", name="myi")
            nc.vector.tensor_mul(myr[:], yr[:], mask1[:])
            nc.vector.tensor_mul(myi[:], yi[:], mask1[:])
            return myr, myi

        my_next = make_my(0)
        pending = None
        for c in range(C):
            myr, myi = my_next
            if c + 1 < C:
                my_next = make_my(c + 1)
            comb = ifft_and_combine(c, myr, myi)
            if pending is not None:
                pending()
            pending = comb
        pending()

        # r0 = p0 = rhs = ap + lam*x ; x0 = 0
        nc.vector.tensor_scalar_mul(scr[:], x_re[:], lam_b[:, 0:1])
        nc.vector.tensor_add(r_re[:], ap_re[:], scr[:])
        nc.vector.tensor_scalar_mul(scr2[:], x_im[:], lam_b[:, 0:1])
        nc.vector.tensor_add(r_im[:], ap_im[:], scr2[:])
        nc.scalar.copy(p_re[:], r_re[:])
        nc.scalar.copy(p_im[:], r_im[:])
        nc.scalar.copy(p16r[:], r_re[:])
        nc.scalar.copy(p16i[:], r_im[:])
        g1 = tmp16.tile([128, 960], F16, tag="mm_t1", name="mm_t1")
        g2 = tmp16.tile([128, 960], F16, tag="mm_t2", name="mm_t2")
        g3 = tmp16.tile([128, 960], F16, tag="mm_t3", name="mm_t3")
        g4 = tmp16.tile([128, 960], F16, tag="mm_t4", name="mm_t4")
        nc.vector.tensor_mul(g1[:], sr[0][:], p16r[:])
        nc.vector.tensor_mul(g2[:], si[0][:], p16i[:])
        nc.vector.tensor_mul(g3[:], sr[0][:], p16i[:])
        nc.vector.tensor_mul(g4[:], si[0][:], p16r[:])
        nc.vector.tensor_sub(sp0r[:], g1[:], g2[:])
        nc.vector.tensor_add(sp0i[:], g3[:], g4[:])
        nc.vector.memset(x_re[:], 0.0)
        nc.vector.memset(x_im[:], 0.0)

        SQ = mybir.ActivationFunctionType.Square
        # rTr0
        nc.scalar.activation(jnk[:], r_re[:], SQ, accum_out=dacc[:, 0:1])
        nc.scalar.activation(jnk[:], r_im[:], SQ, accum_out=dacc[:, 1:2])
        reduce_dacc(0, 2, 7)
        nc.vector.tensor_add(sc[0:1, 0:1], sc[0:1, 7:8], sc[0:1, 8:9])
        nc.vector.reciprocal(sc[0:1, 1:2], sc[0:1, 0:1])

        # ---- phase 2: CG iterations ----
        def cg_iteration():
            nc.vector.tensor_scalar_mul(ap_re[:], p_re[:], lam_b[:, 0:1])
            nc.scalar.mul(ap_im[:], p_im[:], lam_b[:, 0:1])

            sp_next = (sp0r, sp0i)
            pending = None
            for c in range(C):
                spr, spi = sp_next
                s1r = stg16.tile([128, 960], F16, tag="s1r", name="s1r")
                s1i = stg16.tile([128, 960], F16, tag="s1i", name="s1i")

                def consume1(mt, msz, pre, pim):
                    nc.scalar.copy(s1r[0:msz, _mblk(mt)], pre[0:msz, :])
                    nc.scalar.copy(s1i[0:msz, _mblk(mt)], pim[0:msz, :])
                emit_side(spr, spi, "F", consume1, kt_major=(c == 0))

                wr = stg16.tile([128, 960], F16, tag="wr", name="wr")
                wi = stg16.tile([128, 960], F16, tag="wi", name="wi")

                def consume2(mt, msz, pre, pim):
                    nc.vector.tensor_mul(wr[0:msz, _mblk(mt)], pre[0:msz, :],
                                         mask2[0:msz, _mblk(mt)])
                    nc.vector.tensor_mul(wi[0:msz, _mblk(mt)], pim[0:msz, :],
                                         mask2[0:msz, _mblk(mt)])
                emit_side(s1r, s1i, "F", consume2)
                if pending is not None:
                    pending()

                # prepare next coil's SP before this coil's ifft+combine so the
                # DVE stream feeds the PE ahead of the combine chain
                if c + 1 < C:
                    sp_next = make_sp(c + 1)
                pending = ifft_and_combine(c, wr, wi)
            pending()

            # ---- scalar phase ----
            # pAp = <p, ap> -> alpha; explicit rTrNew dot after the r-update
            # (an algebraic recurrence for rTrNew amplifies fp16 conjugacy
            # error ~25x in final output — measured; keep the real dot).
            nc.vector.tensor_mul(scr[:], p_re[:], ap_re[:])
            nc.vector.tensor_mul(scr2[:], p_im[:], ap_im[:])
            CP = mybir.ActivationFunctionType.Copy
            nc.scalar.activation(jnk[:], scr[:], CP, accum_out=dacc[:, 0:1])
            nc.scalar.activation(jnk[:], scr2[:], CP, accum_out=dacc[:, 1:2])
            reduce_dacc(0, 2, 7)
            warm_chain(dacc, 3)
            nc.vector.tensor_add(sc[0:1, 2:3], sc[0:1, 7:8], sc[0:1, 8:9])   # pAp
            nc.vector.reciprocal(sc[0:1, 11:12], sc[0:1, 2:3])
            nc.vector.tensor_mul(sc[0:1, 3:4], sc[0:1, 0:1], sc[0:1, 11:12])  # alpha
            nc.vector.tensor_scalar_mul(sc[0:1, 4:5], sc[0:1, 3:4], -1.0)
            nc.tensor.matmul(pmisc[:, 4:6], ones_row[0:1, :], sc[0:1, 3:5],
                             start=True, stop=True)
            nc.scalar.copy(ab[:, 0:2], pmisc[:, 4:6])
            warm_mm(ab)
            # alpha*p_old for the deferred x-update (ACT, before p overwrite)
            nc.scalar.mul(jnk[:], p_re[:], ab[:, 0:1])
            nc.scalar.mul(scr2[:], p_im[:], ab[:, 0:1])
            # r -= alpha*Ap
            nc.vector.scalar_tensor_tensor(r_re[:], ap_re[:], ab[:, 1:2],
                                           r_re[:], MULT, ADD)
            nc.vector.scalar_tensor_tensor(r_im[:], ap_im[:], ab[:, 1:2],
                                           r_im[:], MULT, ADD)
            warm_mm(r_re)
            # rTrNew = <r, r>
            nc.scalar.activation(jnk[:], r_re[:], SQ, accum_out=dacc[:, 2:3])
            nc.scalar.activation(jnk[:], r_im[:], SQ, accum_out=dacc[:, 3:4])
            reduce_dacc(2, 2, 7)
            warm_chain(dacc, 3)
            nc.vector.tensor_add(sc[0:1, 6:7], sc[0:1, 7:8], sc[0:1, 8:9])   # rTrNew
            nc.vector.tensor_mul(sc[0:1, 5:6], sc[0:1, 6:7], sc[0:1, 1:2])   # beta
            nc.vector.tensor_copy(sc[0:1, 0:1], sc[0:1, 6:7])
            nc.vector.reciprocal(sc[0:1, 1:2], sc[0:1, 6:7])
            nc.tensor.matmul(pmisc[:, 6:7], ones_row[0:1, :], sc[0:1, 5:6],
                             start=True, stop=True)
            nc.scalar.copy(beta_b[:, 0:1], pmisc[:, 6:7])
            warm_chain(beta_b, 3)
            nc.gpsimd.tensor_add(x_re[:], x_re[:], jnk[:])
            nc.gpsimd.tensor_add(x_im[:], x_im[:], scr2[:])
            # fused per block: p-update -> p16 -> coil-0 S*p products, so the
            # vector queue reaches the first cmul block ~2.5us after beta and
            # the kt-major stage-1 matmuls restart the PE early
            f1 = tmp16.tile([128, 960], F16, tag="mm_t1", name="mm_t1")
            f2 = tmp16.tile([128, 960], F16, tag="mm_t2", name="mm_t2")
            f3 = tmp16.tile([128, 960], F16, tag="mm_t3", name="mm_t3")
            f4 = tmp16.tile([128, 960], F16, tag="mm_t4", name="mm_t4")
            for t in range(3):
                b = slice(320 * t, 320 * t + 320)
                nc.vector.scalar_tensor_tensor(p_re[:, b], p_re[:, b],
                                               beta_b[:, 0:1], r_re[:, b],
                                               MULT, ADD)
                nc.vector.scalar_tensor_tensor(p_im[:, b], p_im[:, b],
                                               beta_b[:, 0:1], r_im[:, b],
                                               MULT, ADD)
                nc.scalar.copy(p16r[:, b], p_re[:, b])
                nc.vector.tensor_copy(p16i[:, b], p_im[:, b])
                nc.vector.tensor_mul(f1[:, b], sr[0][:, b], p16r[:, b])
                nc.vector.tensor_mul(f2[:, b], si[0][:, b], p16i[:, b])
                nc.vector.tensor_mul(f3[:, b], sr[0][:, b], p16i[:, b])
                nc.vector.tensor_mul(f4[:, b], si[0][:, b], p16r[:, b])
                nc.vector.tensor_sub(sp0r[:, b], f1[:, b], f2[:, b])
                nc.vector.tensor_add(sp0i[:, b], f3[:, b], f4[:, b])

        dbg = os.environ.get("CG_DEBUG", "")
        if dbg == "rhs":
            nc.scalar.copy(x_re[:], r_re[:])
            nc.scalar.copy(x_im[:], r_im[:])
        elif UNROLL:
            for _ in range(N_ITER):
                cg_iteration()
        else:
            assert N_ITER % 2 == 0
            with tc.For_i(0, N_ITER // 2, 1):
                cg_iteration()
                cg_iteration()

        for t, (s, sz) in enumerate(KT):
            nc.sync.dma_start(d["out"].ap()[0, s:s + sz, :], x_re[0:sz, _mblk(t)])
            nc.sync.dma_start(d["out"].ap()[1, s:s + sz, :], x_im[0:sz, _mblk(t)])

    nc.compile()
    return nc


def kernel(lambdaa, x_re, x_im, y_re, y_im, smaps_re, smaps_im, mask):
    global _PROGRAM
    lambdaa = np.asarray(lambdaa, np.float32)
    arrs = {
        "x_re": x_re, "x_im": x_im, "y_re": y_re, "y_im": y_im,
    }
    arrs = {k: np.ascontiguousarray(np.asarray(v, np.float32))
            for k, v in arrs.items()}
    arrs["s_re"] = np.ascontiguousarray(np.asarray(smaps_re, np.float16))
    arrs["s_im"] = np.ascontiguousarray(np.asarray(smaps_im, np.float16))
    mask = np.ascontiguousarray(np.asarray(mask, np.float32))

    if _PROGRAM is None:
        _PROGRAM = _build_program()
    nc = _PROGRAM

    consts = _build_consts()
    lam_b = np.full((128, 1), float(lambdaa[0]), np.float32)
    in_maps = []
    for i in range(B):
        in_maps.append({
            **{k: v[i] for k, v in arrs.items()},
            "mask": np.ascontiguousarray(mask[i, 0]),
            "lam_b": lam_b,
            **consts,
        })

    res = bass_utils.run_bass_kernel_spmd(nc, in_maps, core_ids=list(range(B)),
                                          trace=TRACE)
    kernel._last_result = res
    out = np.empty((B, H, H, 2), np.float32)
    for i in range(B):
        o = res.results[i]["out"]
        out[i, :, :, 0] = o[0]
        out[i, :, :, 1] = o[1]
    return out
